# revision 54
# baseline (speedup 1.0000x reference)
"""PatternMemory kernel for 8 Trainium2 NeuronCores — multi-queue,
multi-engine accumulate. CoreSim-modeled 102.1us vs 209.6us for the
single-queue baseline; HW rel err 9.9e-03 (gate 2e-2).

Math (B=8, T=1024, C=1024, P=100):
  ctx_h = context @ W1[:C]                   (B, C)
  trg_h = triggers @ W1[C:]                  (P, C)
  h = relu(ctx_h[:,None,:] + trg_h[None,:,:] + b1)
  logits = h @ W2 + b2[0]                    (B, P)
  scores = sigmoid(logits).mean(axis=0)      (P,)
  w = where(scores > 0.5, scores * conf, 0)
  out = attention_scores + 0.1 * einsum("p,pij->ij", w, biases)

Sharding: core r owns rows [128r, 128(r+1)) of the (T, T) plane; no
collectives (full MLP recomputed per core; remote_dma has no routing
support under this runtime, and collective_compute costs ~15us+).

Design:
- DMA is issued from all three DMA-capable queues (SP, Act, Pool);
  each sustains ~512KB/1.58us, so the ~62MB/core input stream takes
  ~70us of queue time each instead of ~200us on one queue.
- acc = sum_p w[p]*bias[p] runs on three engine pipes in parallel,
  sized so they finish together: DVE scalar_tensor_tensor (4 chains),
  PE w*I diag-matmuls accumulating into PSUM (diags built by Act;
  3 segments), Act scale-copy + Pool tensor_add chain. TensorScalarPtr
  is DVE-only in the V3 ISA (Pool rejected at compile), and fp32r
  matmul needs pre-rounded inputs (precision loss) — both verified.
- MLP in transposed mapping: lhsT = 128x128 W1 blocks, rhs = trigT /
  ctxT, so trg_h/ctx_h land PSUM-transposed as [c',p]/[c',b]: no
  phase-b transposes and cheap N=100/N=8 matmuls; the W2 contraction
  reads hT [c', b*P] directly. Weights are ready ~36us; the rotating
  pools buffer the early stream until then.
- Numerics: weights accumulate UN-scaled (w = score*conf*mask exactly
  as the reference) and the 0.1 is applied once in the final
  attn-add, matching the reference op sequence; chains are short.
  This halves |ours - ref| at near-zero outputs where the elementwise
  rel-err gate binds (1.6e-2 -> 9.9e-3 on HW).
- Engine queues execute in order, so instruction emission follows the
  intended per-engine timeline: all MLP relus run on DVE (idle before
  weights anyway), freeing early Act time for its DMA share; Act's
  diag/scale work comes late; weighted ops follow each plane's
  estimated arrival; the stream tail is DVE-heavy (fastest consumer);
  stores/merges run in [128,512] halves to pipeline.
"""

import numpy as np
import bass_rust

from concourse import bass, mybir
from concourse.bass_utils import run_bass_kernel_spmd
from concourse.tile import TileContext
from concourse.masks import make_identity

B, T, C, P = 8, 1024, 1024, 100
NCORES = 8
ROWS = T // NCORES
FP32 = mybir.dt.float32
AF = mybir.ActivationFunctionType
ALU = mybir.AluOpType

SIM_THRESHOLD = 0.5
LAMBDA = 0.1

# accumulation classes: planes consumed by each engine pipe
N_DVE = 51
N_PE = 31
N_AP = 18
assert N_DVE + N_PE + N_AP == P

# bias-plane DMA caps per queue (sum may exceed P: planner assigns by
# earliest-virtual-clock, caps bound the shares)
Q_BIAS = {"sp": 50, "act": 18, "pool": 32}
# W1 slice shares ([128,512] x32)
Q_W1 = {"sp": 6, "act": 10, "pool": 16}
Q_ATTN = {"sp": 4, "act": 2, "pool": 2}
Q_STORE = {"sp": 2, "act": 4, "pool": 2}

POOL_DVE = 16
POOL_PE = 10
POOL_AP = 6
FLIP_DVE = ()  # (dve_rank, new_cls) surgical class flips
W1_REUSE_FROM = 40  # emission index from which class-0 planes may use w1 slots
# planner virtual clocks: per-queue bias start estimate + issue period (us)
PLAN_CLOCKS = {"sp": 7.6, "act": 4.0, "pool": 15.8}
PLAN_RATES = {"sp": 1.58, "act": 2.9, "pool": 1.9}

_NC_CACHE = {}


def _bresenham(counts):
    total = sum(counts)
    acc = [0.0] * len(counts)
    out = []
    for _ in range(total):
        for i, c in enumerate(counts):
            acc[i] += c / total
        j = max(range(len(counts)), key=lambda i: acc[i])
        acc[j] -= 1.0
        out.append(j)
    return out


def _plan():
    """Static schedule: per-plane (class, queue, est arrival)."""
    # Back-load the dve class: DVE consumes fastest, so the stream tail is
    # dve-heavy (8 of the last 12) to avoid end-of-stream starvation while
    # the slower PE/AP pipes finish their last planes earlier.
    TAIL = 12
    tail_dve = 8
    tail_pe = 2
    tail_ap = 2
    head = _bresenham([N_DVE - tail_dve, N_PE - tail_pe, N_AP - tail_ap])
    tail = _bresenham([tail_dve, tail_pe, tail_ap])
    cls_seq = head + tail
    assert len(cls_seq) == P
    # surgical rebalance: flip specific dve-ranked planes to another class
    # without re-dealing the whole (phase-locked) sequence
    for rank, ncls in FLIP_DVE:
        idxs = [i for i, c in enumerate(cls_seq) if c == 0]
        cls_seq[idxs[rank]] = ncls
    # queue virtual clocks (bias-stream start estimates, us)
    clocks = dict(PLAN_CLOCKS)
    rate = dict(PLAN_RATES)
    left = dict(Q_BIAS)
    qname = []
    arrival = []
    for p in range(P):
        avail = [q for q in ("sp", "act", "pool") if left[q] > 0]
        q = min(avail, key=lambda n: clocks[n])
        left[q] -= 1
        clocks[q] += rate[q]
        qname.append(q)
        arrival.append(clocks[q])
    return cls_seq, qname, arrival


def _build_nc() -> bass.Bass:
    nc = bass.Bass("TRN2", target_bir_lowering=False, debug=False,
                   num_devices=NCORES)

    bias_s = nc.dram_tensor("bias_s", (P, ROWS, T), FP32, kind="ExternalInput").ap()
    attn_s = nc.dram_tensor("attn_s", (B, ROWS, T), FP32, kind="ExternalInput").ap()
    # [r, (ci*8+kt)*128 + c] = W1half[kt*128 + r, ci*128 + c]
    w1hi = nc.dram_tensor("w1hi", (128, 8192), FP32, kind="ExternalInput").ap()
    w1lo = nc.dram_tensor("w1lo", (128, 8192), FP32, kind="ExternalInput").ap()
    trigp = nc.dram_tensor("trigp", (128, 8 * P), FP32, kind="ExternalInput").ap()
    ctxp = nc.dram_tensor("ctxp", (128, 8 * B), FP32, kind="ExternalInput").ap()
    b1c = nc.dram_tensor("b1c", (128, 8), FP32, kind="ExternalInput").ap()
    w2r = nc.dram_tensor("w2r", (128, 8), FP32, kind="ExternalInput").ap()
    conf = nc.dram_tensor("conf", (1, P), FP32, kind="ExternalInput").ap()
    b2 = nc.dram_tensor("b2", (1, 1), FP32, kind="ExternalInput").ap()
    out_s = nc.dram_tensor("out_s", (B, ROWS, T), FP32, kind="ExternalOutput").ap()

    cls_seq, qname, arrival = _plan()

    with TileContext(nc) as tc:
        with tc.tile_pool(name="const", bufs=1) as const_pool, \
             tc.tile_pool(name="w1p", bufs=8) as w1_pool, \
             tc.tile_pool(name="hT", bufs=2) as hT_pool, \
             tc.tile_pool(name="cb", bufs=2) as cb_pool, \
             tc.tile_pool(name="dg", bufs=4) as dg_pool, \
             tc.tile_pool(name="scp", bufs=2) as sc_pool, \
             tc.tile_pool(name="small", bufs=1) as small_pool, \
             tc.tile_pool(name="acc", bufs=1) as acc_pool, \
             tc.tile_pool(name="pA", bufs=2, space="PSUM") as psA, \
             tc.tile_pool(name="pC", bufs=2, space="PSUM") as psC, \
             tc.tile_pool(name="pL", bufs=1, space="PSUM") as psL, \
             tc.tile_pool(name="pAcc", bufs=1, space="PSUM") as psAcc, \
             tc.tile_pool(name="pdve", bufs=POOL_DVE) as pool_dve, \
             tc.tile_pool(name="ppe", bufs=POOL_PE) as pool_pe, \
             tc.tile_pool(name="pap", bufs=POOL_AP) as pool_ap:

            Q = {"sp": nc.sync, "act": nc.scalar, "pool": nc.gpsimd}

            # ---------- head: consts + W1 ----------
            trigt = const_pool.tile([128, 8 * P], FP32, tag="trigp", name="trigt")
            Q["sp"].dma_start(out=trigt, in_=trigp)
            ctxt = const_pool.tile([128, 8 * B], FP32, tag="ctxp", name="ctxt")
            Q["act"].dma_start(out=ctxt, in_=ctxp)
            b1t = const_pool.tile([128, 8], FP32, tag="b1c", name="b1t")
            Q["act"].dma_start(out=b1t, in_=b1c)
            w2t = const_pool.tile([128, 8], FP32, tag="w2r", name="w2t")
            Q["act"].dma_start(out=w2t, in_=w2r)
            conft = const_pool.tile([1, P], FP32, tag="conf", name="conft")
            Q["act"].dma_start(out=conft, in_=conf)
            b2t = const_pool.tile([1, 1], FP32, tag="b2", name="b2t")
            Q["act"].dma_start(out=b2t, in_=b2)

            ones = const_pool.tile([1, 128], FP32, tag="ones", name="ones")
            nc.vector.memset(ones, 1.0)
            ident = const_pool.tile([128, 128], FP32, tag="ident", name="ident")
            make_identity(nc, ident)

            # W1: 8 rotating [128,1024] slots; slot (ci*2+half) mod 8 holds
            # both ktg halves of (half, ci). ci 0-3 DMA at the head; ci 4-7
            # are woven into the MLP loop (their slot reuse waits on the
            # ci-4 matmuls, so emitting them early would block queue heads).
            w1_tiles = {}
            w1_qseq = [("sp", "act", "pool")[i] for i in
                       _bresenham([Q_W1["sp"], Q_W1["act"], Q_W1["pool"]])]
            w1_i = [0]

            def emit_w1(ci):
                for half, src in ((0, w1hi), (1, w1lo)):
                    t = w1_pool.tile([128, 1024], FP32, tag="w1",
                                     name=f"w1_{half}_{ci}")
                    w1_tiles[(half, ci)] = t
                    for ktg in range(2):
                        col = (ci * 8 + ktg * 4) * 128
                        q = Q[w1_qseq[w1_i[0] % 32]]
                        w1_i[0] += 1
                        q.dma_start(out=t[:, ktg * 512:(ktg + 1) * 512],
                                    in_=src[:, col:col + 512])

            for ci in range(4):
                emit_w1(ci)

            def w1_slice(half, ci, kt):
                t = w1_tiles[(half, ci)]
                return t[:, kt * 128:(kt + 1) * 128]

            # ---------- bias DMA emission helper ----------
            pools = [pool_dve, pool_pe, pool_ap]
            bias_tile = {}
            emitted = [0]  # planes whose DMA has been emitted so far

            def emit_bias_dma(n=1):
                for _ in range(n):
                    p = emitted[0]
                    if p >= P:
                        return
                    emitted[0] += 1
                    cls = cls_seq[p]
                    pool = pools[cls]
                    tag = f"b{cls}"
                    if cls == 0 and p >= W1_REUSE_FROM and p % 2 == 0:
                        pool, tag = w1_pool, "w1"
                    t = pool.tile([128, T], FP32, tag=tag, name=f"bias{p}")
                    Q[qname[p]].dma_start(out=t, in_=bias_s[p])
                    bias_tile[p] = t

            # ---------- MLP (transposed mapping), bias DMAs woven ----------
            log_a = psL.tile([1, 512], FP32, tag="log_a", name="log_a")
            log_b = psL.tile([1, 288], FP32, tag="log_b", name="log_b")
            for ci in range(8):
                if ci + 4 < 8:
                    emit_w1(ci + 4)
                pt = psA.tile([128, P], FP32, tag="pt", name=f"pt{ci}")
                pc = psC.tile([128, B], FP32, tag="pc", name=f"pc{ci}")
                for kt in range(8):
                    nc.tensor.matmul(pt, lhsT=w1_slice(0, ci, kt),
                                     rhs=trigt[:, kt * P:(kt + 1) * P],
                                     start=(kt == 0), stop=(kt == 7))
                for kt in range(8):
                    nc.tensor.matmul(pc, lhsT=w1_slice(1, ci, kt),
                                     rhs=ctxt[:, kt * B:(kt + 1) * B],
                                     start=(kt == 0), stop=(kt == 7))
                cb1 = cb_pool.tile([128, B], FP32, tag="cb1", name=f"cb1_{ci}")
                nc.vector.tensor_scalar(out=cb1, in0=pc,
                                        scalar1=b1t[:, ci:ci + 1],
                                        scalar2=None, op0=ALU.add)
                # all relus on DVE: it idles pre-weights anyway, and this
                # frees ~10us of early Act time for bias DMA + AP scales
                hT = hT_pool.tile([128, B * P], FP32, tag="hT", name=f"hT{ci}")
                for b in range(B):
                    nc.vector.tensor_scalar(out=hT[:, b * P:(b + 1) * P],
                                            in0=pt, scalar1=cb1[:, b:b + 1],
                                            scalar2=0.0, op0=ALU.add,
                                            op1=ALU.max)
                nc.tensor.matmul(log_a, lhsT=w2t[:, ci:ci + 1],
                                 rhs=hT[:, 0:512],
                                 start=(ci == 0), stop=(ci == 7))
                nc.tensor.matmul(log_b, lhsT=w2t[:, ci:ci + 1],
                                 rhs=hT[:, 512:800],
                                 start=(ci == 0), stop=(ci == 7))
                emit_bias_dma((0, 0, 2, 2, 3, 3, 3, 3)[ci])

            # ---------- scores -> weights ----------
            sig = small_pool.tile([1, B * P], FP32, tag="sig", name="sig")
            nc.scalar.activation(out=sig[:, 0:512], in_=log_a,
                                 func=AF.Sigmoid, bias=b2t[:, 0:1])
            nc.scalar.activation(out=sig[:, 512:800], in_=log_b,
                                 func=AF.Sigmoid, bias=b2t[:, 0:1])
            ssum = small_pool.tile([1, P], FP32, tag="ssum", name="ssum")
            nc.vector.tensor_add(out=ssum, in0=sig[:, 0:P], in1=sig[:, P:2 * P])
            for b in range(2, B):
                nc.vector.tensor_add(out=ssum, in0=ssum,
                                     in1=sig[:, b * P:(b + 1) * P])
            scores = small_pool.tile([1, P], FP32, tag="scores", name="scores")
            nc.vector.tensor_scalar_mul(out=scores, in0=ssum, scalar1=1.0 / B)
            mask = small_pool.tile([1, P], FP32, tag="mask", name="mask")
            nc.vector.tensor_scalar(out=mask, in0=scores, scalar1=SIM_THRESHOLD,
                                    scalar2=None, op0=ALU.is_gt)
            # weights WITHOUT the lambda fold (w = score*conf*mask, exactly
            # the reference's `weights`); 0.1 is applied once at the end so
            # per-plane roundings match the reference op sequence.
            sc_conf = small_pool.tile([1, P], FP32, tag="sc_conf", name="sc_conf")
            nc.vector.tensor_mul(out=sc_conf, in0=scores, in1=conft)
            w_vec = small_pool.tile([1, P], FP32, tag="w_vec", name="w_vec")
            nc.vector.tensor_mul(out=w_vec, in0=sc_conf, in1=mask)
            wbc = psA.tile([128, P], FP32, tag="pt", name="wbc")
            nc.tensor.matmul(wbc, lhsT=ones, rhs=w_vec, start=True, stop=True)
            wsb = small_pool.tile([128, P], FP32, tag="wsb", name="wsb")
            nc.scalar.activation(out=wsb, in_=wbc, func=AF.Copy)
            emit_bias_dma(6)

            # ---------- weighted accumulation, arrival-ordered ----------
            # 4 DVE chains + 3 PE psum segments + 1 Act/Pool chain: short
            # chains keep f32 accumulation error well under the reference's
            # own rounding noise.
            a_dve = [acc_pool.tile([128, T], FP32, tag=f"ad{i}", name=f"ad{i}")
                     for i in range(4)]
            a_ap = acc_pool.tile([128, T], FP32, tag="aap", name="aap")
            acc_ps = psAcc.tile([128, T], FP32, tag="accps", name="accps")
            n_pe_act = cls_seq.count(1)
            PE_SEG = (n_pe_act + 2) // 3

            def flush_pe_segment():
                seg = sc_pool.tile([128, T], FP32, tag="sc", name="peseg")
                nc.scalar.activation(out=seg, in_=acc_ps, func=AF.Copy)
                nc.gpsimd.tensor_add(out=a_ap, in0=a_ap, in1=seg)

            order = sorted(range(P), key=lambda p: arrival[p])
            n_seen = {0: 0, 1: 0, 2: 0}
            for p in order:
                cls = cls_seq[p]
                i = n_seen[cls]
                n_seen[cls] += 1
                t = bias_tile.get(p)
                if t is None:
                    emit_bias_dma(P)  # shouldn't happen; drain
                    t = bias_tile[p]
                w_ap = wsb[:, p:p + 1]
                if cls == 0:      # DVE stt, 4 chains
                    ch = a_dve[i % 4]

                    if i < 4:
                        nc.vector.tensor_scalar_mul(out=ch, in0=t, scalar1=w_ap)
                    else:
                        nc.vector.scalar_tensor_tensor(out=ch, in0=t,
                                                       scalar=w_ap, in1=ch,
                                                       op0=ALU.mult,
                                                       op1=ALU.add)
                elif cls == 1:    # Act diag build + PE matmul accumulate
                    if i > 0 and i % PE_SEG == 0:
                        flush_pe_segment()
                    diag = dg_pool.tile([128, 128], FP32, tag="dg",
                                        name=f"dg{p}")
                    nc.scalar.activation(out=diag, in_=ident, func=AF.Copy,
                                         scale=w_ap)
                    seg_i = i % PE_SEG
                    seg_last = (seg_i == PE_SEG - 1) or (i == n_pe_act - 1)
                    for h in range(2):
                        nc.tensor.matmul(acc_ps[:, h * 512:(h + 1) * 512],
                                         lhsT=diag,
                                         rhs=t[:, h * 512:(h + 1) * 512],
                                         start=(seg_i == 0), stop=seg_last)
                else:             # Act scale + Pool add
                    if i == 0:
                        nc.scalar.activation(out=a_ap, in_=t, func=AF.Copy,
                                             scale=w_ap)
                    else:
                        sc = sc_pool.tile([128, T], FP32, tag="sc",
                                          name=f"sc{p}")
                        nc.scalar.activation(out=sc, in_=t, func=AF.Copy,
                                             scale=w_ap)
                        nc.gpsimd.tensor_add(out=a_ap, in0=a_ap, in1=sc)
                emit_bias_dma(1)

            # ---------- attn stream + merge + tail ----------
            attn_qseq = [("sp", "act", "pool")[i] for i in
                         _bresenham([Q_ATTN["sp"], Q_ATTN["act"], Q_ATTN["pool"]])]
            attns = []
            for b in range(B):
                t = w1_pool.tile([128, T], FP32, tag="w1", name=f"attn{b}")
                Q[attn_qseq[b]].dma_start(out=t, in_=attn_s[b])
                attns.append(t)

            # merge + tail in [128,512] halves so the first half's adds and
            # stores overlap the second half's merge. tot is the raw
            # (un-lambda'd) weighted sum; out_b = attn_b + 0.1*tot matches
            # the reference's final op sequence.
            a_pes = sc_pool.tile([128, T], FP32, tag="sc", name="apes")
            st_qseq = [("sp", "act", "pool")[i] for i in
                       _bresenham([Q_STORE["sp"], Q_STORE["act"], Q_STORE["pool"]])]
            tot = a_dve[0]
            t01 = a_dve[1]  # freed after the first merge below
            for h in range(2):
                hs = slice(h * 512, (h + 1) * 512)
                nc.scalar.activation(out=a_pes[:, hs], in_=acc_ps[:, hs],
                                     func=AF.Copy)
                nc.vector.tensor_add(out=a_dve[0][:, hs], in0=a_dve[0][:, hs],
                                     in1=a_dve[1][:, hs])
                nc.vector.tensor_add(out=a_dve[2][:, hs], in0=a_dve[2][:, hs],
                                     in1=a_dve[3][:, hs])
                nc.gpsimd.tensor_add(out=a_ap[:, hs], in0=a_ap[:, hs],
                                     in1=a_pes[:, hs])
                nc.vector.tensor_add(out=a_dve[0][:, hs], in0=a_dve[0][:, hs],
                                     in1=a_dve[2][:, hs])
                nc.vector.tensor_add(out=a_dve[0][:, hs], in0=a_dve[0][:, hs],
                                     in1=a_ap[:, hs])
                # Pool lacks immediate-scalar fused ops we trust on V3, so
                # prescale 0.1*tot once on DVE for Pool's adds; DVE's own
                # adds fuse the 0.1 into a scalar_tensor_tensor.
                nc.vector.tensor_scalar_mul(out=t01[:, hs], in0=tot[:, hs],
                                            scalar1=LAMBDA)
                for b in range(B):
                    if b in (0, 2, 4, 6):
                        nc.vector.scalar_tensor_tensor(out=attns[b][:, hs],
                                                       in0=tot[:, hs],
                                                       scalar=LAMBDA,
                                                       in1=attns[b][:, hs],
                                                       op0=ALU.mult,
                                                       op1=ALU.add)
                    else:
                        nc.gpsimd.tensor_add(out=attns[b][:, hs],
                                             in0=attns[b][:, hs],
                                             in1=t01[:, hs])
                    Q[st_qseq[b]].dma_start(out=out_s[b][:, hs],
                                            in_=attns[b][:, hs])

    bass_rust.generate_event_semaphores(nc)
    return nc


def _get_nc() -> bass.Bass:
    if "nc" not in _NC_CACHE:
        _NC_CACHE["nc"] = _build_nc()
    return _NC_CACHE["nc"]


def _prep_in_maps(attention_scores, context, triggers, biases, confidences,
                  W1, b1, W2, b2):
    f32 = np.float32
    W1 = np.asarray(W1, dtype=f32)
    # [r, (ci*8+kt)*128 + c] = W1half[kt*128 + r, ci*128 + c]
    w1hi_h = np.ascontiguousarray(
        W1[C:].reshape(8, 128, 8, 128).transpose(1, 2, 0, 3).reshape(128, 8192))
    w1lo_h = np.ascontiguousarray(
        W1[:C].reshape(8, 128, 8, 128).transpose(1, 2, 0, 3).reshape(128, 8192))
    trigp_h = np.ascontiguousarray(
        np.asarray(triggers, dtype=f32).T.reshape(8, 128, P)
        .transpose(1, 0, 2).reshape(128, 8 * P))
    ctxp_h = np.ascontiguousarray(
        np.asarray(context, dtype=f32).T.reshape(8, 128, B)
        .transpose(1, 0, 2).reshape(128, 8 * B))
    b1c_h = np.ascontiguousarray(np.asarray(b1, dtype=f32).reshape(8, 128).T)
    w2r_h = np.ascontiguousarray(np.asarray(W2, dtype=f32).reshape(8, 128).T)
    conf_h = np.ascontiguousarray(np.asarray(confidences, dtype=f32).reshape(1, P))
    b2_h = np.ascontiguousarray(np.asarray(b2, dtype=f32).reshape(1, 1))
    attention_scores = np.asarray(attention_scores, dtype=f32)
    biases = np.asarray(biases, dtype=f32)
    in_maps = []
    for r in range(NCORES):
        rows = slice(r * ROWS, (r + 1) * ROWS)
        in_maps.append({
            "bias_s": np.ascontiguousarray(biases[:, rows, :]),
            "attn_s": np.ascontiguousarray(attention_scores[:, rows, :]),
            "w1hi": w1hi_h,
            "w1lo": w1lo_h,
            "trigp": trigp_h,
            "ctxp": ctxp_h,
            "b1c": b1c_h,
            "w2r": w2r_h,
            "conf": conf_h,
            "b2": b2_h,
        })
    return in_maps


def run(trace=False, **inputs):
    nc = _get_nc()
    in_maps = _prep_in_maps(**inputs)
    try:
        res = run_bass_kernel_spmd(nc, in_maps, core_ids=list(range(NCORES)),
                                   trace=trace)
    except ModuleNotFoundError:
        # trace=True needs the axon NTFF profile hook, absent in this
        # client; fall back to an untraced run.
        res = run_bass_kernel_spmd(nc, in_maps, core_ids=list(range(NCORES)),
                                   trace=False)
    out = np.concatenate([np.asarray(res.results[r]["out_s"])
                          for r in range(NCORES)], axis=1)
    return out.astype(np.float32), res


def kernel(**inputs) -> np.ndarray:
    out, _ = run(trace=False, **inputs)
    return out
